# revision 29
# baseline (speedup 1.0000x reference)
"""AVR render kernel for 8 trn2 NeuronCores.

Math: the reference's per-(b,ray,sample) rfft is linear, so the ray/sample
reduction commutes with it:

  out[b,k] = sum_s phase[s,k] * rfft_l(env[s,l] * acc[b,s,l])[k]
  acc[b,s,l] = sum_r w[b,r,s] * (l >= d[b,r,s]) * signal[b,r,s,l]

with w the volume-rendering weights, d the tx-delay threshold, env the
tail-mask * path-loss envelope (s,l only). Only `acc` touches the 302 MB
signal tensor -> that's the device kernel (rays sharded across 8 cores);
everything downstream is [2,64,512]-sized and finishes on host in f64.

Device layout (v2): rays on partitions. Per (b, s): DVE applies the delay
mask in-place via one scalar_tensor_tensor (msig = (iota >= d) * sig), then
PE does the weight-multiply + ray-reduction as a transposed matmul
(out[l_chunk, 1] = msig.T @ w) into PSUM columns. The tail mask makes
env[s, l] = 0 for l >= u[s], so only columns [0, u[s]) are ever loaded or
processed (58% of the bytes). Host finishes env * rfft * phase * sum_s in
float64 on the gathered [128, npc] partials.
"""

import os

import ml_dtypes
import numpy as np

os.environ.setdefault("MYCRO_LOCAL_CACHE", "1")

# Problem constants (hardcoded per contract)
N_AZI = 48
N_ELE = 24
N_SAMPLES = 64
NEAR = 0.2
FAR = 10.0
SPEED = 343.0
FS = 16000.0
PATHLOSS = 1.0
XYZ_MIN = -5.0
XYZ_MAX = 5.0
N_RAYS = N_AZI * N_ELE + 2  # 1154
L = 512
LH = L // 2 + 1  # 257
BS = 2

N_CORES = 8
RPC = 145  # rays per core (8*145 = 1160 >= 1154, zero-weight padded)
R1 = 128  # chunk1 rays (matmul K=128, partition base 0)
R2 = RPC - R1  # 17 chunk2 rays, packed 3 s per tile at partition bases 0/32/64

LAST_RESULT = None  # BassKernelResults of the most recent device run
RUN_KWARGS = {}  # extra kwargs for run_bass_kernel_spmd (test harness hooks)


# --------------------------------------------------------------------------
# Geometry: the tail mask zeroes env[s, l] for l >= u[s], so only columns
# [0, u[s]) of each (s) row ever matter — value-independent pruning.
# --------------------------------------------------------------------------
def _geometry(lo=None):
    """Layout geometry. `lo` is an optional [BS, 64] int array of per-(b,s)
    lower column bounds (global min delay over all rays) — columns below it
    are exactly zero after masking, columns >= u[s] are killed by the tail
    mask on host. Active range per (b, s): [lo[b,s], u[s])."""
    f32 = np.float32
    d_vals = (np.linspace(0.0, 1.0, N_SAMPLES, dtype=f32) * f32(FAR - NEAR)) + f32(NEAR)
    pts2rx = (f32(FS) * d_vals / f32(SPEED)).astype(f32)
    shift = np.round(pts2rx).astype(np.int64)
    u = (L - 1 - shift).astype(np.int64)  # [64] usable width per s
    if lo is None:
        lo = np.zeros((BS, N_SAMPLES), np.int64)
    lo = np.minimum(lo.astype(np.int64), u[None, :])  # clamp; empty -> width 0

    buckets = [(8 * k, 8 * k + 8) for k in range(8)]
    groups = [(3 * g, min(3 * g + 3, N_SAMPLES)) for g in range(22)]

    # chunk1: per (b, bucket): per-s slot offsets in a packed tile
    c1 = {}  # (b,k) -> dict(offs={s: off}, widths={s: w}, total)
    for b in range(BS):
        for k, (a, e) in enumerate(buckets):
            offs, widths, tot = {}, {}, 0
            for s in range(a, e):
                w = int(max(0, u[s] - lo[b, s]))
                offs[s], widths[s] = tot, w
                tot += w
            c1[(b, k)] = {"offs": offs, "widths": widths, "total": tot}

    # chunk2: per (b, group): uniform col range [glo, ghi)
    c2 = {}  # (b,gi) -> (glo, ghi)
    for b in range(BS):
        for gi, (a, e) in enumerate(groups):
            glo = int(min(lo[b, s] for s in range(a, e)))
            ghi = int(max(u[s] for s in range(a, e)))
            c2[(b, gi)] = (glo, max(glo, ghi))

    # psum column map: (b, s) -> list of (col, labs, wj) with absolute l range
    cols = {}
    ncol = 0
    for b in range(BS):
        for s in range(N_SAMPLES):
            lst = []
            pos = int(lo[b, s])
            while pos < int(u[s]):
                wj = min(int(u[s]) - pos, 128)
                lst.append((ncol, pos, wj))
                ncol += 1
                pos += wj
            cols[(b, s)] = lst
    return {
        "u": u,
        "lo": lo,
        "shift": shift,
        "pts2rx": pts2rx,
        "buckets": buckets,
        "groups": groups,
        "c1": c1,
        "c2": c2,
        "cols": cols,
        "npc": ncol,
    }


# --------------------------------------------------------------------------
# Bass program (identical on all 8 cores; per-core data differs)
# --------------------------------------------------------------------------
_PROGRAM = None
_PROGRAM_KEY = None


def _split_multi_waits(nc):
    """This walrus build supports one sync-wait per instruction; Tile emits
    several. Hoist extras onto single-wait Drain carriers."""
    import concourse.mybir as mybir

    n = 0
    for f in nc.m.functions:
        for b in f.blocks:
            new = []
            for ins in b.instructions:
                si = getattr(ins, "sync_info", None)
                waits = list(si.on_wait) if si is not None and si.on_wait else []
                if len(waits) > 1:
                    for w in waits[:-1]:
                        n += 1
                        d = mybir.InstDrain(
                            name=f"waitsplit-{n}", ins=[], outs=[], bass_is_fusable=False
                        )
                        d.engine = ins.engine
                        d.sync_info = mybir.SyncInfo(on_wait=[w], on_update=[])
                        new.append(d)
                    si.on_wait = [waits[-1]]
                new.append(ins)
            b.instructions = new
    return n


def _build_program(geo):
    global _PROGRAM, _PROGRAM_KEY
    key = geo["lo"].tobytes()
    if _PROGRAM is not None and _PROGRAM_KEY == key:
        return _PROGRAM

    import concourse.bass as bass
    import concourse.mybir as mybir
    from concourse.tile import TileContext

    u, lo, buckets, groups, c1, c2, cols, npc = (
        geo["u"], geo["lo"], geo["buckets"], geo["groups"], geo["c1"], geo["c2"],
        geo["cols"], geo["npc"],
    )
    f32 = mybir.dt.float32
    bf16 = mybir.dt.bfloat16
    ig = mybir.AluOpType.is_ge
    mu = mybir.AluOpType.mult
    NCOL = 128 + BS * len(groups)

    nc = bass.Bass(
        "TRN2",
        target_bir_lowering=False,
        debug=False,
        enable_asserts=False,
        num_devices=N_CORES,
    )
    sig1 = {}
    sig2 = {}
    for b in range(BS):
        for k in range(len(buckets)):
            tot = c1[(b, k)]["total"]
            if tot > 0:
                sig1[(b, k)] = nc.dram_tensor(
                    f"sig1_{b}_{k}", [R1, tot], f32, kind="ExternalInput"
                )
        for gi, (a, e) in enumerate(groups):
            glo, ghi = c2[(b, gi)]
            if ghi > glo and any(len(cols[(b, s)]) > 0 for s in range(a, e)):
                sig2[(b, gi)] = nc.dram_tensor(
                    f"sig2_{b}_{gi}", [32 * (e - a), ghi - glo], f32,
                    kind="ExternalInput",
                )
    dcol = nc.dram_tensor("dcol", [128, NCOL], f32, kind="ExternalInput")
    wcol = nc.dram_tensor("wcol", [128, NCOL], bf16, kind="ExternalInput")
    iod = nc.dram_tensor("iod", [128, L], f32, kind="ExternalInput")
    outp = nc.dram_tensor("outp", [128, npc], f32, kind="ExternalOutput")

    with TileContext(nc) as tc:
        with (
            tc.tile_pool(name="const", bufs=1) as cpool,
            tc.tile_pool(name="s1p", bufs=6) as p1,
            tc.tile_pool(name="s2p", bufs=6) as p2,
            tc.tile_pool(name="m1p", bufs=4) as m1p,
            tc.tile_pool(name="m2p", bufs=4) as m2p,
            tc.tile_pool(name="psum", bufs=1, space="PSUM") as pp,
        ):
            d_t = cpool.tile([128, NCOL], f32, tag="dcol")
            w_t = cpool.tile([128, NCOL], bf16, tag="wcol")
            io_t = cpool.tile([128, L], f32, tag="iod")
            osb = cpool.tile([128, npc], f32, tag="osb")
            pacc = pp.tile([128, 512], f32, tag="pacc")
            pacc2 = pp.tile([128, 512], f32, tag="pacc2")
            nc.sync.dma_start(d_t[:], dcol.ap())
            nc.scalar.dma_start(w_t[:], wcol.ap())
            nc.scalar.dma_start(io_t[:], iod.ap())

            def emit_c1(b, k, dma_eng):
                meta = c1[(b, k)]
                tot = meta["total"]
                t1 = p1.tile([R1, tot], f32, tag="sig1")
                ms1 = m1p.tile([R1, tot], bf16, tag="msig1")
                dma_eng.dma_start(t1[:, :tot], sig1[(b, k)].ap())
                for s in range(*buckets[k]):
                    w = meta["widths"][s]
                    if w == 0:
                        continue
                    off = meta["offs"][s]
                    l0 = int(lo[b, s])
                    c = b * N_SAMPLES + s
                    nc.vector.scalar_tensor_tensor(
                        ms1[:, off : off + w],
                        io_t[:, l0 : l0 + w],
                        d_t[:, c : c + 1],
                        t1[:, off : off + w],
                        ig,
                        mu,
                    )
                    for col, labs, wj in cols[(b, s)]:
                        o = off + (labs - l0)
                        nc.tensor.matmul(
                            pacc[0:wj, col : col + 1],
                            ms1[:, o : o + wj],
                            w_t[:, c : c + 1],
                            start=True,
                            stop=True,
                        )

            def emit_c2(b, gi, dma_eng):
                a, e = groups[gi]
                glo, ghi = c2[(b, gi)]
                wg = ghi - glo
                t2 = p2.tile([128, wg], f32, tag="sig2")
                ms2 = m2p.tile([128, wg], bf16, tag="msig2")
                dma_eng.dma_start(t2[0 : 32 * (e - a), :], sig2[(b, gi)].ap())
                c2c = 128 + b * len(groups) + gi
                nc.vector.scalar_tensor_tensor(
                    ms2[:],
                    io_t[:, glo : glo + wg],
                    d_t[:, c2c : c2c + 1],
                    t2[:],
                    ig,
                    mu,
                )
                for sl in range(e - a):
                    s = a + sl
                    base = 32 * sl
                    for col, labs, wj in cols[(b, s)]:
                        o = labs - glo
                        nc.tensor.matmul(
                            pacc2[0:wj, col : col + 1],
                            ms2[base : base + R2, o : o + wj],
                            w_t[base : base + R2, c2c : c2c + 1],
                            start=True,
                            stop=True,
                        )

            jobs = []
            for b in range(BS):
                for k in range(len(buckets)):
                    if c1[(b, k)]["total"] > 0:
                        jobs.append((c1[(b, k)]["total"] * R1, "c1", b, k))
                for gi in range(len(groups)):
                    if (b, gi) in sig2:
                        a, e = groups[gi]
                        glo, ghi = c2[(b, gi)]
                        jobs.append((32 * (e - a) * (ghi - glo), "c2", b, gi))
            jobs.sort()
            rings = [nc.sync, nc.scalar, nc.gpsimd]
            for ji, (_, kind, b, idx) in enumerate(jobs):
                dma_eng = rings[ji % 3]
                if kind == "c1":
                    emit_c1(b, idx, dma_eng)
                else:
                    emit_c2(b, idx, dma_eng)
            nc.vector.tensor_copy(osb[:], pacc[:, 0:npc])
            nc.vector.tensor_tensor(
                osb[:], osb[:], pacc2[:, 0:npc], mybir.AluOpType.add
            )
            nc.sync.dma_start(outp.ap(), osb[:])

    _split_multi_waits(nc)
    _PROGRAM = nc
    _PROGRAM_KEY = key
    return nc


# --------------------------------------------------------------------------
# Host-side math (pure numpy; f32 op order mirrors reference.py where
# rounding matters). _AZI_JITTER_BITS = exact f32 bits of
# jax.random.uniform(jax.random.key(42), (48,)) — fixed constant of the
# reference's deterministic ray_directions.
# --------------------------------------------------------------------------
_AZI_JITTER_BITS = np.array(
    [
        1058541776, 1054884672, 1059121640, 1006739840, 1049527168, 1061957808,
        1062872700, 1064724584, 1059926876, 1038314304, 1056830152, 1060842556,
        1044529808, 1042161728, 1025229728, 1051448012, 1064149566, 1058850320,
        1056717136, 1060887222, 1044496352, 1042124560, 1024990240, 1051236868,
        1064299654, 1058571048, 1055675880, 1059809482, 1032831424, 1053410564,
        1057935414, 1052069980, 1064544354, 1059213386, 1003863040, 1049412292,
        1061804220, 1062657056, 1063830886, 1058333210, 1054422212, 1058830422,
        1056673896, 1061050176, 1046852432, 1046224544, 1043272896, 1033101952,
    ],
    dtype=np.uint32,
)


def _ray_directions_f32():
    jitter = _AZI_JITTER_BITS.view(np.float32)
    azi = np.linspace(0.0, 2.0 * np.pi, N_AZI + 1, dtype=np.float32)[:-1]
    azi = (azi + np.float32(2.0 * np.pi / N_AZI) * jitter).astype(np.float32)
    ele = np.linspace(0.0, 1.0, N_ELE + 2, dtype=np.float32)[1:-1]
    ele = np.arccos((np.float32(2.0) * ele - np.float32(1.0)).astype(np.float32))
    A, E = np.meshgrid(azi, ele, indexing="ij")
    a, e = A.flatten().astype(np.float32), E.flatten().astype(np.float32)
    d = np.stack(
        [np.cos(a) * np.sin(e), np.sin(a) * np.sin(e), np.cos(e)], axis=-1
    ).astype(np.float32)
    d = np.concatenate(
        [d, np.array([[0.0, 0.0, 1.0], [0.0, 0.0, -1.0]], dtype=np.float32)], axis=0
    )
    return d  # [1154, 3] f32


def _host_prep(rays_o, position_tx, attn):
    """Returns (delay [2,1154,64] f32, weights [2,1154,64] f32,
    env [64,512] f64, phase [64,257] c128)."""
    f32 = np.float32
    direc = _ray_directions_f32()
    d_vals = (np.linspace(0.0, 1.0, N_SAMPLES, dtype=f32) * f32(FAR - NEAR)) + f32(NEAR)

    # denorm(norm(tx) - norm(pts)) == tx - pts up to f32 rounding; mirror the
    # reference's op order exactly so round() boundaries agree.
    def norm_p(p):
        return (f32(2.0) * (p - f32(XYZ_MIN)) / f32(XYZ_MAX - XYZ_MIN) - f32(1.0)).astype(f32)

    def denorm_p(p):
        return ((p + f32(1.0)) / f32(2.0) * f32(XYZ_MAX - XYZ_MIN) + f32(XYZ_MIN)).astype(f32)

    ray_pts = (
        rays_o[:, None, None, :].astype(f32)
        + direc[None, :, None, :] * d_vals[None, None, :, None]
    ).astype(f32)
    network_pts = norm_p(ray_pts)
    network_tx = norm_p(position_tx.astype(f32))[:, None, None, :]
    diff = denorm_p((network_tx - network_pts).astype(f32))
    tx2pts_idx = (
        np.sqrt((diff.astype(f32) ** 2).sum(axis=-1, dtype=f32)).astype(f32)
        * f32(FS)
        / f32(SPEED)
    ).astype(f32)
    delay = np.clip(np.round(tx2pts_idx), 0, L - 1).astype(f32)  # [2,1154,64]

    pts2rx_idx = (f32(FS) * d_vals / f32(SPEED)).astype(f32)  # [64] unrounded
    shift = np.round(pts2rx_idx).astype(np.int64)  # [64]

    dists = np.concatenate([d_vals[1:] - d_vals[:-1], np.array([1e10], dtype=f32)])
    alpha = (f32(1.0) - np.exp(-attn.astype(f32) * dists[None, None, :])).astype(f32)
    att_i = np.cumprod(
        np.concatenate(
            [np.ones_like(alpha[..., :1]), (f32(1.0) - alpha + f32(1e-6)).astype(f32)],
            axis=-1,
        ),
        axis=-1,
        dtype=f32,
    )[..., :-1]
    weights = (att_i * alpha).astype(f32)  # [2,1154,64]

    # envelope (s,l): tail mask * path loss gather (exact integer indexing)
    tail_mask = (np.arange(L - 1, -1, -1)[None, :] - shift[:, None]) > 0
    prev_part = int(0.1 / SPEED * FS)
    ideal = np.arange(int(L * 2.5), dtype=np.float64) / FS * SPEED
    pl = PATHLOSS / (ideal + 0.001)
    pl[:prev_part] = pl[prev_part + 1]
    pl_all = pl[shift[:, None] + np.arange(L)[None, :]]  # [64,512]
    env = tail_mask.astype(np.float64) * pl_all

    phase = np.exp(
        -1j
        * 2.0
        * np.pi
        / L
        * np.arange(LH)[None, :]
        * pts2rx_idx.astype(np.float64)[:, None]
    )  # [64,257]
    return delay, weights, env, phase


def _shard_inputs(geo, signal, delay, weights):
    """Per-core input maps for the pruned rays-on-partitions layout."""
    u, lo, buckets, groups, c1, c2 = (
        geo["u"], geo["lo"], geo["buckets"], geo["groups"], geo["c1"], geo["c2"],
    )
    ngr = len(groups)
    ncol = 128 + BS * ngr
    iod = np.ascontiguousarray(
        np.broadcast_to(np.arange(L, dtype=np.float32), (128, L))
    )
    in_maps = []
    for c in range(N_CORES):
        rr = np.arange(c * RPC, (c + 1) * RPC)
        valid = rr < N_RAYS
        rrc = np.clip(rr, 0, N_RAYS - 1)
        m = {"iod": iod}
        for b in range(BS):
            sb1 = signal[b, rrc[0:R1]]  # [128, 64, 512]
            sb2 = signal[b, rrc[R1:RPC]]  # [17, 64, 512]
            for k, (a, e) in enumerate(buckets):
                meta = c1[(b, k)]
                tot = meta["total"]
                if tot == 0:
                    continue
                arr = np.empty((R1, tot), np.float32)
                for s in range(a, e):
                    w = meta["widths"][s]
                    if w == 0:
                        continue
                    off = meta["offs"][s]
                    l0 = int(lo[b, s])
                    arr[:, off : off + w] = sb1[:, s, l0 : l0 + w]
                m[f"sig1_{b}_{k}"] = arr
            for gi, (a, e) in enumerate(groups):
                glo, ghi = c2[(b, gi)]
                wg = ghi - glo
                if wg == 0 or all(
                    len(geo["cols"][(b, s)]) == 0 for s in range(a, e)
                ):
                    continue
                arr = np.zeros((32 * (e - a), wg), np.float32)
                for sl in range(e - a):
                    arr[32 * sl : 32 * sl + R2] = sb2[:, a + sl, glo:ghi]
                m[f"sig2_{b}_{gi}"] = arr
        dc = np.zeros((128, ncol), np.float32)
        wc = np.zeros((128, ncol), np.float32)
        d1 = delay[:, rrc[0:R1], :]  # [2, 128, 64]
        w1 = weights[:, rrc[0:R1], :] * valid[None, 0:R1, None]
        d2 = delay[:, rrc[R1:RPC], :]  # [2, 17, 64]
        w2 = weights[:, rrc[R1:RPC], :] * valid[None, R1:RPC, None]
        for b in range(BS):
            dc[:, b * N_SAMPLES : (b + 1) * N_SAMPLES] = d1[b]
            wc[:, b * N_SAMPLES : (b + 1) * N_SAMPLES] = w1[b].astype(np.float32)
            for gi, (a, e) in enumerate(groups):
                cc = 128 + b * ngr + gi
                for sl in range(e - a):
                    dc[32 * sl : 32 * sl + R2, cc] = d2[b, :, a + sl]
                    wc[32 * sl : 32 * sl + R2, cc] = w2[b, :, a + sl]
        m["dcol"] = dc
        m["wcol"] = wc.astype(ml_dtypes.bfloat16)
        in_maps.append(m)
    return in_maps


def kernel(rays_o, position_tx, attn, signal):
    global LAST_RESULT
    from concourse import bass_utils

    rays_o = np.asarray(rays_o, dtype=np.float32)
    position_tx = np.asarray(position_tx, dtype=np.float32)
    attn = np.asarray(attn, dtype=np.float32)
    signal = np.asarray(signal, dtype=np.float32)

    delay, weights, env, phase = _host_prep(rays_o, position_tx, attn)
    lo = delay[:, :N_RAYS, :].min(axis=1).astype(np.int64)  # [2, 64]
    geo = _geometry(lo)
    in_maps = _shard_inputs(geo, signal, delay, weights)
    nc = _build_program(geo)

    br = bass_utils.run_bass_kernel_spmd(
        nc, in_maps, core_ids=list(range(N_CORES)), **RUN_KWARGS
    )
    LAST_RESULT = br

    parts = np.stack([r["outp"] for r in br.results])  # [8, 128, npc]
    total = parts.astype(np.float64).sum(axis=0)  # [128, npc]
    acc = np.zeros((BS, N_SAMPLES, L), np.float64)
    for (b, s), lst in geo["cols"].items():
        for col, labs, wj in lst:
            acc[b, s, labs : labs + wj] = total[0:wj, col]
    spec = np.fft.rfft(acc * env[None], axis=-1)  # [2,64,257]
    rec = (spec * phase[None]).sum(axis=1)  # [2,257]
    return np.stack([rec.real, rec.imag], axis=-1).astype(np.float32)


# revision 30
# speedup vs baseline: 1.1800x; 1.1800x over previous
"""AVR render kernel for 8 trn2 NeuronCores.

Math: the reference's per-(b,ray,sample) rfft is linear, so the ray/sample
reduction commutes with it:

  out[b,k] = sum_s phase[s,k] * rfft_l(env[s,l] * acc[b,s,l])[k]
  acc[b,s,l] = sum_r w[b,r,s] * (l >= d[b,r,s]) * signal[b,r,s,l]

with w the volume-rendering weights, d the tx-delay threshold, env the
tail-mask * path-loss envelope (s,l only). Only `acc` touches the 302 MB
signal tensor -> that's the device kernel (rays sharded across 8 cores);
everything downstream is [2,64,512]-sized and finishes on host in f64.

Device layout (v2): rays on partitions. Per (b, s): DVE applies the delay
mask in-place via one scalar_tensor_tensor (msig = (iota >= d) * sig), then
PE does the weight-multiply + ray-reduction as a transposed matmul
(out[l_chunk, 1] = msig.T @ w) into PSUM columns. The tail mask makes
env[s, l] = 0 for l >= u[s], so only columns [0, u[s]) are ever loaded or
processed (58% of the bytes). Host finishes env * rfft * phase * sum_s in
float64 on the gathered [128, npc] partials.
"""

import os

import ml_dtypes
import numpy as np

os.environ.setdefault("MYCRO_LOCAL_CACHE", "1")

# Problem constants (hardcoded per contract)
N_AZI = 48
N_ELE = 24
N_SAMPLES = 64
NEAR = 0.2
FAR = 10.0
SPEED = 343.0
FS = 16000.0
PATHLOSS = 1.0
XYZ_MIN = -5.0
XYZ_MAX = 5.0
N_RAYS = N_AZI * N_ELE + 2  # 1154
L = 512
LH = L // 2 + 1  # 257
BS = 2

N_CORES = 8
RPC = 145  # rays per core (8*145 = 1160 >= 1154, zero-weight padded)
R1 = 128  # chunk1 rays (matmul K=128, partition base 0)
R2 = RPC - R1  # 17 chunk2 rays, packed 3 s per tile at partition bases 0/32/64

LAST_RESULT = None  # BassKernelResults of the most recent device run
RUN_KWARGS = {}  # extra kwargs for run_bass_kernel_spmd (test harness hooks)


# --------------------------------------------------------------------------
# Geometry: the tail mask zeroes env[s, l] for l >= u[s], so only columns
# [0, u[s]) of each (s) row ever matter — value-independent pruning.
# --------------------------------------------------------------------------
def _geometry(lo=None):
    """Layout geometry. `lo` is an optional [BS, 64] int array of per-(b,s)
    lower column bounds (global min delay over all rays) — columns below it
    are exactly zero after masking, columns >= u[s] are killed by the tail
    mask on host. Active range per (b, s): [lo[b,s], u[s])."""
    f32 = np.float32
    d_vals = (np.linspace(0.0, 1.0, N_SAMPLES, dtype=f32) * f32(FAR - NEAR)) + f32(NEAR)
    pts2rx = (f32(FS) * d_vals / f32(SPEED)).astype(f32)
    shift = np.round(pts2rx).astype(np.int64)
    u = (L - 1 - shift).astype(np.int64)  # [64] usable width per s
    if lo is None:
        lo = np.zeros((BS, N_SAMPLES), np.int64)
    lo = np.minimum(lo.astype(np.int64), u[None, :])  # clamp; empty -> width 0

    buckets = [(8 * k, 8 * k + 8) for k in range(8)]
    groups = [(3 * g, min(3 * g + 3, N_SAMPLES)) for g in range(22)]

    # chunk1: per (b, bucket): per-s slot offsets in a packed tile
    c1 = {}  # (b,k) -> dict(offs={s: off}, widths={s: w}, total)
    for b in range(BS):
        for k, (a, e) in enumerate(buckets):
            offs, widths, tot = {}, {}, 0
            for s in range(a, e):
                w = int(max(0, u[s] - lo[b, s]))
                offs[s], widths[s] = tot, w
                tot += w
            c1[(b, k)] = {"offs": offs, "widths": widths, "total": tot}

    # chunk2: per (b, group): uniform col range [glo, ghi)
    c2 = {}  # (b,gi) -> (glo, ghi)
    for b in range(BS):
        for gi, (a, e) in enumerate(groups):
            glo = int(min(lo[b, s] for s in range(a, e)))
            ghi = int(max(u[s] for s in range(a, e)))
            c2[(b, gi)] = (glo, max(glo, ghi))

    # psum column map: (b, s) -> list of (col, labs, wj) with absolute l range
    cols = {}
    ncol = 0
    for b in range(BS):
        for s in range(N_SAMPLES):
            lst = []
            pos = int(lo[b, s])
            while pos < int(u[s]):
                wj = min(int(u[s]) - pos, 128)
                lst.append((ncol, pos, wj))
                ncol += 1
                pos += wj
            cols[(b, s)] = lst
    return {
        "u": u,
        "lo": lo,
        "shift": shift,
        "pts2rx": pts2rx,
        "buckets": buckets,
        "groups": groups,
        "c1": c1,
        "c2": c2,
        "cols": cols,
        "npc": ncol,
    }


# --------------------------------------------------------------------------
# Bass program (identical on all 8 cores; per-core data differs)
# --------------------------------------------------------------------------
_PROGRAM = None
_PROGRAM_KEY = None


def _split_multi_waits(nc):
    """This walrus build supports one sync-wait per instruction; Tile emits
    several. Hoist extras onto single-wait Drain carriers."""
    import concourse.mybir as mybir

    n = 0
    for f in nc.m.functions:
        for b in f.blocks:
            new = []
            for ins in b.instructions:
                si = getattr(ins, "sync_info", None)
                waits = list(si.on_wait) if si is not None and si.on_wait else []
                if len(waits) > 1:
                    for w in waits[:-1]:
                        n += 1
                        d = mybir.InstDrain(
                            name=f"waitsplit-{n}", ins=[], outs=[], bass_is_fusable=False
                        )
                        d.engine = ins.engine
                        d.sync_info = mybir.SyncInfo(on_wait=[w], on_update=[])
                        new.append(d)
                    si.on_wait = [waits[-1]]
                new.append(ins)
            b.instructions = new
    return n


def _build_program(geo):
    global _PROGRAM, _PROGRAM_KEY
    key = geo["lo"].tobytes()
    if _PROGRAM is not None and _PROGRAM_KEY == key:
        return _PROGRAM

    import concourse.bass as bass
    import concourse.mybir as mybir
    from concourse.tile import TileContext

    u, lo, buckets, groups, c1, c2, cols, npc = (
        geo["u"], geo["lo"], geo["buckets"], geo["groups"], geo["c1"], geo["c2"],
        geo["cols"], geo["npc"],
    )
    f32 = mybir.dt.float32
    bf16 = mybir.dt.bfloat16
    ig = mybir.AluOpType.is_ge
    mu = mybir.AluOpType.mult
    NCOL = 128 + BS * len(groups)

    nc = bass.Bass(
        "TRN2",
        target_bir_lowering=False,
        debug=False,
        enable_asserts=False,
        num_devices=N_CORES,
    )
    sig1 = {}
    sig2 = {}
    for b in range(BS):
        for k in range(len(buckets)):
            tot = c1[(b, k)]["total"]
            if tot > 0:
                sig1[(b, k)] = nc.dram_tensor(
                    f"sig1_{b}_{k}", [R1, tot], f32, kind="ExternalInput"
                )
        for gi, (a, e) in enumerate(groups):
            glo, ghi = c2[(b, gi)]
            if ghi > glo and any(len(cols[(b, s)]) > 0 for s in range(a, e)):
                sig2[(b, gi)] = nc.dram_tensor(
                    f"sig2_{b}_{gi}", [32 * (e - a), ghi - glo], f32,
                    kind="ExternalInput",
                )
    dcol = nc.dram_tensor("dcol", [128, NCOL], f32, kind="ExternalInput")
    wcol = nc.dram_tensor("wcol", [128, NCOL], bf16, kind="ExternalInput")
    iod = nc.dram_tensor("iod", [128, L], f32, kind="ExternalInput")
    outp = nc.dram_tensor("outp", [128, npc], f32, kind="ExternalOutput")

    with TileContext(nc) as tc:
        with (
            tc.tile_pool(name="const", bufs=1) as cpool,
            tc.tile_pool(name="s1p", bufs=6) as p1,
            tc.tile_pool(name="s2p", bufs=6) as p2,
            tc.tile_pool(name="m1p", bufs=4) as m1p,
            tc.tile_pool(name="m2p", bufs=4) as m2p,
            tc.tile_pool(name="psum", bufs=1, space="PSUM") as pp,
        ):
            d_t = cpool.tile([128, NCOL], f32, tag="dcol")
            w_t = cpool.tile([128, NCOL], bf16, tag="wcol")
            io_t = cpool.tile([128, L], f32, tag="iod")
            osb = cpool.tile([128, npc], f32, tag="osb")
            pacc = pp.tile([128, 512], f32, tag="pacc")
            pacc2 = pp.tile([128, 512], f32, tag="pacc2")
            nc.sync.dma_start(d_t[:], dcol.ap())
            nc.scalar.dma_start(w_t[:], wcol.ap())
            nc.scalar.dma_start(io_t[:], iod.ap())

            def emit_c1(b, k, dma_eng):
                meta = c1[(b, k)]
                tot = meta["total"]
                t1 = p1.tile([R1, tot], f32, tag="sig1")
                ms1 = m1p.tile([R1, tot], bf16, tag="msig1")
                dma_eng.dma_start(t1[:, :tot], sig1[(b, k)].ap())
                for s in range(*buckets[k]):
                    w = meta["widths"][s]
                    if w == 0:
                        continue
                    off = meta["offs"][s]
                    l0 = int(lo[b, s])
                    c = b * N_SAMPLES + s
                    nc.vector.scalar_tensor_tensor(
                        ms1[:, off : off + w],
                        io_t[:, l0 : l0 + w],
                        d_t[:, c : c + 1],
                        t1[:, off : off + w],
                        ig,
                        mu,
                    )
                    for col, labs, wj in cols[(b, s)]:
                        o = off + (labs - l0)
                        nc.tensor.matmul(
                            pacc[0:wj, col : col + 1],
                            ms1[:, o : o + wj],
                            w_t[:, c : c + 1],
                            start=True,
                            stop=True,
                        )

            def emit_c2(b, gi, dma_eng):
                a, e = groups[gi]
                glo, ghi = c2[(b, gi)]
                wg = ghi - glo
                t2 = p2.tile([128, wg], f32, tag="sig2")
                ms2 = m2p.tile([128, wg], bf16, tag="msig2")
                dma_eng.dma_start(t2[0 : 32 * (e - a), :], sig2[(b, gi)].ap())
                c2c = 128 + b * len(groups) + gi
                nc.vector.scalar_tensor_tensor(
                    ms2[:],
                    io_t[:, glo : glo + wg],
                    d_t[:, c2c : c2c + 1],
                    t2[:],
                    ig,
                    mu,
                )
                for sl in range(e - a):
                    s = a + sl
                    base = 32 * sl
                    for col, labs, wj in cols[(b, s)]:
                        o = labs - glo
                        nc.tensor.matmul(
                            pacc2[0:wj, col : col + 1],
                            ms2[base : base + R2, o : o + wj],
                            w_t[base : base + R2, c2c : c2c + 1],
                            start=True,
                            stop=True,
                        )

            jobs = []
            for b in range(BS):
                for k in range(len(buckets)):
                    if c1[(b, k)]["total"] > 0:
                        jobs.append((c1[(b, k)]["total"] * R1, "c1", b, k))
                for gi in range(len(groups)):
                    if (b, gi) in sig2:
                        a, e = groups[gi]
                        glo, ghi = c2[(b, gi)]
                        jobs.append((32 * (e - a) * (ghi - glo), "c2", b, gi))
            jobs.sort()
            rings = [nc.sync, nc.scalar]
            for ji, (_, kind, b, idx) in enumerate(jobs):
                dma_eng = rings[ji % 2]
                if kind == "c1":
                    emit_c1(b, idx, dma_eng)
                else:
                    emit_c2(b, idx, dma_eng)
            nc.vector.tensor_copy(osb[:], pacc[:, 0:npc])
            nc.vector.tensor_tensor(
                osb[:], osb[:], pacc2[:, 0:npc], mybir.AluOpType.add
            )
            nc.sync.dma_start(outp.ap(), osb[:])

    _split_multi_waits(nc)
    _PROGRAM = nc
    _PROGRAM_KEY = key
    return nc


# --------------------------------------------------------------------------
# Host-side math (pure numpy; f32 op order mirrors reference.py where
# rounding matters). _AZI_JITTER_BITS = exact f32 bits of
# jax.random.uniform(jax.random.key(42), (48,)) — fixed constant of the
# reference's deterministic ray_directions.
# --------------------------------------------------------------------------
_AZI_JITTER_BITS = np.array(
    [
        1058541776, 1054884672, 1059121640, 1006739840, 1049527168, 1061957808,
        1062872700, 1064724584, 1059926876, 1038314304, 1056830152, 1060842556,
        1044529808, 1042161728, 1025229728, 1051448012, 1064149566, 1058850320,
        1056717136, 1060887222, 1044496352, 1042124560, 1024990240, 1051236868,
        1064299654, 1058571048, 1055675880, 1059809482, 1032831424, 1053410564,
        1057935414, 1052069980, 1064544354, 1059213386, 1003863040, 1049412292,
        1061804220, 1062657056, 1063830886, 1058333210, 1054422212, 1058830422,
        1056673896, 1061050176, 1046852432, 1046224544, 1043272896, 1033101952,
    ],
    dtype=np.uint32,
)


def _ray_directions_f32():
    jitter = _AZI_JITTER_BITS.view(np.float32)
    azi = np.linspace(0.0, 2.0 * np.pi, N_AZI + 1, dtype=np.float32)[:-1]
    azi = (azi + np.float32(2.0 * np.pi / N_AZI) * jitter).astype(np.float32)
    ele = np.linspace(0.0, 1.0, N_ELE + 2, dtype=np.float32)[1:-1]
    ele = np.arccos((np.float32(2.0) * ele - np.float32(1.0)).astype(np.float32))
    A, E = np.meshgrid(azi, ele, indexing="ij")
    a, e = A.flatten().astype(np.float32), E.flatten().astype(np.float32)
    d = np.stack(
        [np.cos(a) * np.sin(e), np.sin(a) * np.sin(e), np.cos(e)], axis=-1
    ).astype(np.float32)
    d = np.concatenate(
        [d, np.array([[0.0, 0.0, 1.0], [0.0, 0.0, -1.0]], dtype=np.float32)], axis=0
    )
    return d  # [1154, 3] f32


def _host_prep(rays_o, position_tx, attn):
    """Returns (delay [2,1154,64] f32, weights [2,1154,64] f32,
    env [64,512] f64, phase [64,257] c128)."""
    f32 = np.float32
    direc = _ray_directions_f32()
    d_vals = (np.linspace(0.0, 1.0, N_SAMPLES, dtype=f32) * f32(FAR - NEAR)) + f32(NEAR)

    # denorm(norm(tx) - norm(pts)) == tx - pts up to f32 rounding; mirror the
    # reference's op order exactly so round() boundaries agree.
    def norm_p(p):
        return (f32(2.0) * (p - f32(XYZ_MIN)) / f32(XYZ_MAX - XYZ_MIN) - f32(1.0)).astype(f32)

    def denorm_p(p):
        return ((p + f32(1.0)) / f32(2.0) * f32(XYZ_MAX - XYZ_MIN) + f32(XYZ_MIN)).astype(f32)

    ray_pts = (
        rays_o[:, None, None, :].astype(f32)
        + direc[None, :, None, :] * d_vals[None, None, :, None]
    ).astype(f32)
    network_pts = norm_p(ray_pts)
    network_tx = norm_p(position_tx.astype(f32))[:, None, None, :]
    diff = denorm_p((network_tx - network_pts).astype(f32))
    tx2pts_idx = (
        np.sqrt((diff.astype(f32) ** 2).sum(axis=-1, dtype=f32)).astype(f32)
        * f32(FS)
        / f32(SPEED)
    ).astype(f32)
    delay = np.clip(np.round(tx2pts_idx), 0, L - 1).astype(f32)  # [2,1154,64]

    pts2rx_idx = (f32(FS) * d_vals / f32(SPEED)).astype(f32)  # [64] unrounded
    shift = np.round(pts2rx_idx).astype(np.int64)  # [64]

    dists = np.concatenate([d_vals[1:] - d_vals[:-1], np.array([1e10], dtype=f32)])
    alpha = (f32(1.0) - np.exp(-attn.astype(f32) * dists[None, None, :])).astype(f32)
    att_i = np.cumprod(
        np.concatenate(
            [np.ones_like(alpha[..., :1]), (f32(1.0) - alpha + f32(1e-6)).astype(f32)],
            axis=-1,
        ),
        axis=-1,
        dtype=f32,
    )[..., :-1]
    weights = (att_i * alpha).astype(f32)  # [2,1154,64]

    # envelope (s,l): tail mask * path loss gather (exact integer indexing)
    tail_mask = (np.arange(L - 1, -1, -1)[None, :] - shift[:, None]) > 0
    prev_part = int(0.1 / SPEED * FS)
    ideal = np.arange(int(L * 2.5), dtype=np.float64) / FS * SPEED
    pl = PATHLOSS / (ideal + 0.001)
    pl[:prev_part] = pl[prev_part + 1]
    pl_all = pl[shift[:, None] + np.arange(L)[None, :]]  # [64,512]
    env = tail_mask.astype(np.float64) * pl_all

    phase = np.exp(
        -1j
        * 2.0
        * np.pi
        / L
        * np.arange(LH)[None, :]
        * pts2rx_idx.astype(np.float64)[:, None]
    )  # [64,257]
    return delay, weights, env, phase


def _shard_inputs(geo, signal, delay, weights):
    """Per-core input maps for the pruned rays-on-partitions layout."""
    u, lo, buckets, groups, c1, c2 = (
        geo["u"], geo["lo"], geo["buckets"], geo["groups"], geo["c1"], geo["c2"],
    )
    ngr = len(groups)
    ncol = 128 + BS * ngr
    iod = np.ascontiguousarray(
        np.broadcast_to(np.arange(L, dtype=np.float32), (128, L))
    )
    in_maps = []
    for c in range(N_CORES):
        rr = np.arange(c * RPC, (c + 1) * RPC)
        valid = rr < N_RAYS
        rrc = np.clip(rr, 0, N_RAYS - 1)
        m = {"iod": iod}
        for b in range(BS):
            sb1 = signal[b, rrc[0:R1]]  # [128, 64, 512]
            sb2 = signal[b, rrc[R1:RPC]]  # [17, 64, 512]
            for k, (a, e) in enumerate(buckets):
                meta = c1[(b, k)]
                tot = meta["total"]
                if tot == 0:
                    continue
                arr = np.empty((R1, tot), np.float32)
                for s in range(a, e):
                    w = meta["widths"][s]
                    if w == 0:
                        continue
                    off = meta["offs"][s]
                    l0 = int(lo[b, s])
                    arr[:, off : off + w] = sb1[:, s, l0 : l0 + w]
                m[f"sig1_{b}_{k}"] = arr
            for gi, (a, e) in enumerate(groups):
                glo, ghi = c2[(b, gi)]
                wg = ghi - glo
                if wg == 0 or all(
                    len(geo["cols"][(b, s)]) == 0 for s in range(a, e)
                ):
                    continue
                arr = np.zeros((32 * (e - a), wg), np.float32)
                for sl in range(e - a):
                    arr[32 * sl : 32 * sl + R2] = sb2[:, a + sl, glo:ghi]
                m[f"sig2_{b}_{gi}"] = arr
        dc = np.zeros((128, ncol), np.float32)
        wc = np.zeros((128, ncol), np.float32)
        d1 = delay[:, rrc[0:R1], :]  # [2, 128, 64]
        w1 = weights[:, rrc[0:R1], :] * valid[None, 0:R1, None]
        d2 = delay[:, rrc[R1:RPC], :]  # [2, 17, 64]
        w2 = weights[:, rrc[R1:RPC], :] * valid[None, R1:RPC, None]
        for b in range(BS):
            dc[:, b * N_SAMPLES : (b + 1) * N_SAMPLES] = d1[b]
            wc[:, b * N_SAMPLES : (b + 1) * N_SAMPLES] = w1[b].astype(np.float32)
            for gi, (a, e) in enumerate(groups):
                cc = 128 + b * ngr + gi
                for sl in range(e - a):
                    dc[32 * sl : 32 * sl + R2, cc] = d2[b, :, a + sl]
                    wc[32 * sl : 32 * sl + R2, cc] = w2[b, :, a + sl]
        m["dcol"] = dc
        m["wcol"] = wc.astype(ml_dtypes.bfloat16)
        in_maps.append(m)
    return in_maps


def kernel(rays_o, position_tx, attn, signal):
    global LAST_RESULT
    from concourse import bass_utils

    rays_o = np.asarray(rays_o, dtype=np.float32)
    position_tx = np.asarray(position_tx, dtype=np.float32)
    attn = np.asarray(attn, dtype=np.float32)
    signal = np.asarray(signal, dtype=np.float32)

    delay, weights, env, phase = _host_prep(rays_o, position_tx, attn)
    lo = delay[:, :N_RAYS, :].min(axis=1).astype(np.int64)  # [2, 64]
    geo = _geometry(lo)
    in_maps = _shard_inputs(geo, signal, delay, weights)
    nc = _build_program(geo)

    br = bass_utils.run_bass_kernel_spmd(
        nc, in_maps, core_ids=list(range(N_CORES)), **RUN_KWARGS
    )
    LAST_RESULT = br

    parts = np.stack([r["outp"] for r in br.results])  # [8, 128, npc]
    total = parts.astype(np.float64).sum(axis=0)  # [128, npc]
    acc = np.zeros((BS, N_SAMPLES, L), np.float64)
    for (b, s), lst in geo["cols"].items():
        for col, labs, wj in lst:
            acc[b, s, labs : labs + wj] = total[0:wj, col]
    spec = np.fft.rfft(acc * env[None], axis=-1)  # [2,64,257]
    rec = (spec * phase[None]).sum(axis=1)  # [2,257]
    return np.stack([rec.real, rec.imag], axis=-1).astype(np.float32)
